# revision 22
# baseline (speedup 1.0000x reference)
"""Trainium2 Bass kernel for nn_RadialModel (forward NUFFT, radial MRI).

Per-core (1 frame, all 8 coils):
  1. coil multiply (DVE, bf16 out)       cimage = (xr+ixi)*(cr+ici)
  2. DFT via PE bf16 matmuls (2 stages): G[v,u] = A @ (M^T @ A^T) with
     apodization + fftshift phases folded into the constant A matrices;
     accumulated per v-tile into stg[vt][v, u, cri(16)] bf16
  3. y-tap-interleaved table build: T2[r, q, s(5), cri(16)] bf16 in DRAM,
     T2[r,q,s] = G[(r+s-2) mod 512, (q-2) mod 512]. The s-shifts are done
     with PE matmuls against shifted-identity matrices (main + edge part
     from the neighbouring v-tile), drained straight into interleaved
     row-chunk tiles, then written with large contiguous DMAs.
  4. interpolation: 5x5 tap window centred on round(g) (the dropped 6th
     reference tap has weight < 2.5e-3).  ONE indirect-DMA descriptor per
     point fetches the whole 5-cell x 80-element 800B patch (the SWDGE
     ucode supports exactly one index per partition per call -> 128 calls
     instead of 768).  Weighted reduce on DVE.
  5. sqrt(w) scale + store

Sharding: one frame (nt) per NeuronCore, 8 cores. Host does only
shard/reshape/unshuffle; all math on device.
"""
import numpy as np

import concourse.bass as bass
import concourse.bacc as bacc
import concourse.mybir as mybir
import concourse.tile as tile
from concourse.bass_utils import run_bass_kernel_spmd
from concourse.masks import make_identity

F32 = mybir.dt.float32
I32 = mybir.dt.int32
BF16 = mybir.dt.bfloat16
AX = mybir.AxisListType
OP = mybir.AluOpType

IM = 256
G = 512
J = 6              # Kaiser-Bessel width of the reference kernel
ALPHA = 2.34 * J
TWO_PI = 2.0 * np.pi
NT, NC, K = 8, 8, 16384
TAPS = 5           # tap window per dim (centre 5 of the 6 reference taps)
CRI = NC * 2       # 16 interleaved coil-re/im values
CELL = TAPS * CRI  # 80 elements per (r, q) table cell
QC = 517           # table cells per row: 2 left halo + 512 + 3 right
ROWS = G
NTILE = 16         # point tiles of 1024 points (8 groups x 128 partitions)
GRP = 8
DEG = 8            # KB weight polynomial degree; abs err ~8.5e-6


# ---------------------------------------------------------------- host consts
def _host_consts():
    # apodization correction 1/FT(kb)
    f = (np.arange(IM) - IM // 2) / G
    z = (np.pi * J * f) ** 2 - ALPHA ** 2
    s = np.sqrt(np.abs(z))
    val = np.where(z < 0, np.sinh(s) / np.maximum(s, 1e-12), np.sinc(s / np.pi))
    ftkb = (J / np.i0(ALPHA)) * val
    scal = 1.0 / ftkb
    # A[u, x'] = e^{i pi u/2 - 2 pi i u x'/G} * scal[x'] / sqrt(G)
    u = np.arange(G)[:, None].astype(np.float64)
    xp = np.arange(IM)[None, :].astype(np.float64)
    A = np.exp(1j * np.pi * u / 2 - 2j * np.pi * u * xp / G) * scal[None, :] / np.sqrt(G)
    art = np.ascontiguousarray(A.T.real, dtype=np.float32)   # [256, 512]
    ait = np.ascontiguousarray(A.T.imag, dtype=np.float32)
    aitn = np.ascontiguousarray(-A.T.imag, dtype=np.float32)
    # polynomial fit of w(t) = i0(ALPHA*sqrt(t))/i0(ALPHA) on t in [0,1]
    n = 512
    x = (1 - np.cos(np.pi * (np.arange(n) + 0.5) / n)) / 2
    w = np.i0(ALPHA * np.sqrt(x)) / np.i0(ALPHA)
    V = np.vander(x, DEG + 1, increasing=True)
    c, *_ = np.linalg.lstsq(V, w, rcond=None)
    # shifted identities [v, r]: main M_d (v = r+d) and edge E_d from the
    # neighbouring v-tile, for d in (-2, -1, 1, 2) -> slots (2i, 2i+1)
    sh = np.zeros((8, 128, 128), np.float32)
    for i, d in enumerate((-2, -1, 1, 2)):
        sh[2 * i] = np.eye(128, k=-d, dtype=np.float32)
        sh[2 * i + 1] = np.eye(128, k=(128 - d) if d > 0 else (-128 - d),
                               dtype=np.float32)
    return art, ait, aitn, c.astype(np.float64), \
        np.ascontiguousarray(sh.reshape(8 * 128, 128))


_ART, _AIT, _AITN, _CHEB, _SHIFTS = _host_consts()


# ---------------------------------------------------------------- bass build
def build_bass(debug=False):
    nc = bacc.Bacc()

    x_in = nc.declare_dram_parameter("x", [2, IM, IM], F32, isOutput=False)
    k_in = nc.declare_dram_parameter("kk", [2, K], F32, isOutput=False)
    c_in = nc.declare_dram_parameter("coil", [NC, 2, IM, IM], F32, isOutput=False)
    w_in = nc.declare_dram_parameter("wr", [128, NTILE * 128], F32, isOutput=False)
    art_in = nc.declare_dram_parameter("art", [IM, G], F32, isOutput=False)
    ait_in = nc.declare_dram_parameter("ait", [IM, G], F32, isOutput=False)
    aitn_in = nc.declare_dram_parameter("aitn", [IM, G], F32, isOutput=False)
    sh_in = nc.declare_dram_parameter("shifts", [8 * 128, 128], F32, isOutput=False)
    y_out = nc.declare_dram_parameter("yr", [128, NTILE * 128], F32, isOutput=True)

    T2 = nc.dram_tensor("T2", [ROWS, QC * CELL], BF16)

    CH = _CHEB
    with tile.TileContext(nc) as tc:
        with (
            tc.tile_pool(name="const", bufs=1) as constp,
            tc.tile_pool(name="work", bufs=1) as workp,
            tc.tile_pool(name="ctile", bufs=2) as coilp,
            tc.tile_pool(name="mtile", bufs=4) as mp,
            tc.tile_pool(name="bt", bufs=6) as btp,
            tc.tile_pool(name="stg", bufs=1) as stgp,
            tc.tile_pool(name="rowc", bufs=2) as rowcp,
            tc.tile_pool(name="patch", bufs=2) as patchp,
            tc.tile_pool(name="wp", bufs=2) as wpp,
            tc.tile_pool(name="ps1", bufs=4, space="PSUM") as ps1,
            tc.tile_pool(name="ps2", bufs=2, space="PSUM") as ps2,
        ):
            # ---------------- constants ----------------
            ident = constp.tile([128, 128], F32, tag="ident")
            make_identity(nc, ident[:])
            fstg = workp.tile([128, G], F32, tag="fstg")
            art = []
            for name, src in (("art", art_in), ("ait", ait_in), ("aitn", aitn_in)):
                ts_ = []
                for xt in range(2):
                    nc.sync.dma_start(
                        out=fstg[:], in_=src[xt * 128:(xt + 1) * 128, :])
                    t_ = constp.tile([128, G], BF16, tag=f"{name}{xt}")
                    nc.vector.tensor_copy(out=t_[:], in_=fstg[:])
                    ts_.append(t_)
                art.append(ts_)
            artT, aitT, aitnT = art

            # shifted identities (bf16), d=(-2,-1,1,2); edge rows fixed via
            # small direct DRAM writes instead of edge matmuls
            shm = {}
            for i, d in enumerate((-2, -1, 1, 2)):
                nc.sync.dma_start(
                    out=fstg[:, 0:128],
                    in_=sh_in[(2 * i) * 128:(2 * i + 1) * 128, :])
                tm = constp.tile([128, 128], BF16, tag=f"shm{i}")
                nc.vector.tensor_copy(out=tm[:], in_=fstg[:, 0:128])
                shm[d] = tm

            offs = constp.tile([128, TAPS], F32, tag="offs")
            for a in range(TAPS):
                nc.vector.memset(offs[:, a:a + 1], float(2 - a))

            # ---------------- k -> [p, c] transpose ----------------
            kg = workp.tile([128, 256], F32, tag="kg")  # [p, (d, c)]
            for d in range(2):
                kt_in = workp.tile([128, 128], F32, tag="ktin")
                nc.sync.dma_start(
                    out=kt_in[:], in_=k_in[d].rearrange("(c p) -> c p", p=128)
                )
                ktp = ps2.tile([128, 128], F32, tag="psb")
                nc.tensor.transpose(ktp[:], kt_in[:], ident[:])
                nc.scalar.copy(out=kg[:, d * 128:(d + 1) * 128], in_=ktp[:])

            # ---------------- w load + sqrt ----------------
            wsq = workp.tile([128, NTILE * 128], F32, tag="wsq")
            nc.sync.dma_start(out=wsq[:], in_=w_in[:])
            nc.scalar.activation(
                out=wsq[:], in_=wsq[:],
                func=mybir.ActivationFunctionType.Sqrt,
            )

            # ---------------- index & weight math (DVE) ----------------
            # gxy = om*(G/2pi) mod G  -> [0, 512)
            gxy = workp.tile([128, 256], F32, tag="gxy")
            nc.vector.tensor_scalar_mul(gxy[:], kg[:], float(G / TWO_PI))
            msk = workp.tile([128, 256], F32, tag="msk")
            nc.vector.tensor_scalar(
                out=msk[:], in0=gxy[:], scalar1=0.0, scalar2=None, op0=OP.is_lt
            )
            nc.vector.scalar_tensor_tensor(
                out=gxy[:], in0=msk[:], scalar=float(G), in1=gxy[:],
                op0=OP.mult, op1=OP.add,
            )
            # fl = rne(gxy) via 2^23 trick ; f = gxy - fl in [-0.5, 0.5]
            fl = workp.tile([128, 256], F32, tag="fl")
            nc.vector.tensor_scalar(
                out=fl[:], in0=gxy[:], scalar1=12582912.0, scalar2=12582912.0,
                op0=OP.add, op1=OP.subtract,
            )
            ff = workp.tile([128, 256], F32, tag="ff")
            nc.vector.tensor_sub(ff[:], gxy[:], fl[:])
            # flm = fl mod 512  (fl in [0, 512])
            nc.vector.tensor_scalar(
                out=msk[:], in0=fl[:], scalar1=511.5, scalar2=None, op0=OP.is_gt
            )
            flm = workp.tile([128, 256], F32, tag="flm")
            nc.vector.scalar_tensor_tensor(
                out=flm[:], in0=msk[:], scalar=float(-G), in1=fl[:],
                op0=OP.mult, op1=OP.add,
            )

            # U[p, (dc, j)] = f + (2 - j)
            ut = workp.tile([128, 256 * TAPS], F32, tag="ut")
            ut3 = ut[:].rearrange("p (dc j) -> p dc j", j=TAPS)
            nc.vector.tensor_tensor(
                out=ut3,
                in0=ff[:].unsqueeze(2).broadcast_to([128, 256, TAPS]),
                in1=offs[:].unsqueeze(1).broadcast_to([128, 256, TAPS]),
                op=OP.add,
            )
            # t = 1 - (U/3)^2   (in-place square then affine)
            nc.vector.tensor_mul(ut[:], ut[:], ut[:])
            nc.vector.tensor_scalar(
                out=ut[:], in0=ut[:], scalar1=float(-1.0 / 9.0), scalar2=1.0,
                op0=OP.mult, op1=OP.add,
            )
            # Horner in t
            acc = workp.tile([128, 256 * TAPS], F32, tag="acc")
            nc.vector.tensor_scalar(
                out=acc[:], in0=ut[:], scalar1=float(CH[DEG]),
                scalar2=float(CH[DEG - 1]), op0=OP.mult, op1=OP.add,
            )
            for dd in range(DEG - 2, -1, -1):
                nc.vector.tensor_mul(acc[:], acc[:], ut[:])
                nc.vector.tensor_scalar_add(acc[:], acc[:], float(CH[dd]))
            # acc = [p, (d, c, j)]: d=0 -> wx taps, d=1 -> wy taps

            # gather cell index: idx = ry*517 + rx  (r = flm)
            fy517 = workp.tile([128, 128], F32, tag="fy517")
            nc.vector.tensor_scalar_mul(fy517[:], flm[:, 128:256], float(QC))
            idxf = workp.tile([128, 128], F32, tag="idxf")
            nc.vector.tensor_add(idxf[:], fy517[:], flm[:, 0:128])
            idx32 = workp.tile([128, 128], I32, tag="idx32")
            nc.vector.tensor_copy(out=idx32[:], in_=idxf[:])

            # ---------------- res buffer (bf16: 2x reduce rate) ----------------
            res = workp.tile([128, NTILE * 128], BF16, tag="res")

            # precompute all per-tile tap-weight products (vector is idle
            # during the DFT phase); w25b[p, (t, g, a, b)] bf16
            w25all = workp.tile([128, NTILE * GRP * TAPS * TAPS], BF16,
                                tag="w25all")
            w25f = workp.tile([128, GRP * TAPS * TAPS], F32, tag="w25f")
            for t in range(NTILE):
                w253 = w25f[:].rearrange("p (g a b) -> p g a b", a=TAPS, b=TAPS)
                wxs = acc[:, t * 40:(t + 1) * 40].rearrange(
                    "p (g a) -> p g a", a=TAPS)
                wys = acc[:, 640 + t * 40: 640 + (t + 1) * 40].rearrange(
                    "p (g b) -> p g b", b=TAPS)
                nc.vector.tensor_tensor(
                    out=w253,
                    in0=wxs.unsqueeze(3).broadcast_to([128, GRP, TAPS, TAPS]),
                    in1=wys.unsqueeze(2).broadcast_to([128, GRP, TAPS, TAPS]),
                    op=OP.mult,
                )
                nc.vector.tensor_copy(
                    out=w25all[:, t * 200:(t + 1) * 200], in_=w25f[:])

            drain_flip = [0]

            def _drain(out_ap, in_ap):
                # alternate drains across scalar/vector to balance engines
                if drain_flip[0] % 2 == 0:
                    nc.scalar.copy(out=out_ap, in_=in_ap)
                else:
                    nc.vector.tensor_copy(out=out_ap, in_=in_ap)
                drain_flip[0] += 1

            # x image tiles (persist across all coils), f32
            xts = []
            for xt in range(2):
                xt_t = workp.tile([128, 2 * IM], F32, tag=f"xt{xt}")
                nc.sync.dma_start(
                    out=xt_t[:],
                    in_=x_in[:, xt * 128:(xt + 1) * 128, :]
                    .rearrange("ri x y -> x ri y"),
                )
                xts.append(xt_t)

            # 4 persistent bf16 stagings [v, (u, cri)], filled across coils
            stgs = []
            for vt in range(4):
                stg = stgp.tile([128, G * CRI], BF16, tag=f"stg{vt}")
                stgs.append(stg)

            # =============== phase A: per-coil DFT ===============
            for c in range(NC):
                # ---- coil multiply (bf16 out) ----
                mt = []
                for xt in range(2):
                    ct = coilp.tile([128, 2 * IM], F32, tag="ct")
                    nc.sync.dma_start(
                        out=ct[:],
                        in_=c_in[c, :, xt * 128:(xt + 1) * 128, :]
                        .rearrange("ri x y -> x ri y"),
                    )
                    xt_t = xts[xt]
                    m = mp.tile([128, 2 * IM], BF16, tag="m")
                    xr, xi = xt_t[:, 0:IM], xt_t[:, IM:2 * IM]
                    cr, ci = ct[:, 0:IM], ct[:, IM:2 * IM]
                    mr, mi = m[:, 0:IM], m[:, IM:2 * IM]
                    t1 = mp.tile([128, IM], F32, tag="cm1")
                    t2 = mp.tile([128, IM], F32, tag="cm2")
                    nc.vector.tensor_mul(t1[:], xr, cr)
                    nc.vector.tensor_mul(t2[:], xi, ci)
                    nc.vector.tensor_sub(mr, t1[:], t2[:])
                    nc.vector.tensor_mul(t1[:], xr, ci)
                    nc.vector.tensor_mul(t2[:], xi, cr)
                    nc.vector.tensor_add(mi, t1[:], t2[:])
                    mt.append(m)
                # ---- stage 1: BT[y, u] per (ri, yt), bf16 ----
                bt = {}
                for yt in range(2):
                    pr = ps1.tile([128, G], F32, tag="psa")
                    pi = ps1.tile([128, G], F32, tag="psa")
                    for xt in range(2):
                        mrb = mt[xt][:, yt * 128:yt * 128 + 128]
                        mib = mt[xt][:, IM + yt * 128:IM + yt * 128 + 128]
                        st = xt == 0
                        sp = xt == 1
                        nc.tensor.matmul(pr[:], mrb, artT[xt][:], start=st, stop=False)
                        nc.tensor.matmul(pi[:], mrb, aitT[xt][:], start=st, stop=False)
                        nc.tensor.matmul(pr[:], mib, aitnT[xt][:], start=False, stop=sp)
                        nc.tensor.matmul(pi[:], mib, artT[xt][:], start=False, stop=sp)
                    btr = btp.tile([128, G], BF16, tag="bt")
                    bti = btp.tile([128, G], BF16, tag="bt")
                    _drain(btr[:], pr[:])
                    _drain(bti[:], pi[:])
                    bt[(0, yt)] = btr
                    bt[(1, yt)] = bti
                # ---- stage 2: G[v, u] -> stg[vt] cri slot ----
                for vt in range(4):
                    stg3 = stgs[vt][:].rearrange("p (u e) -> p u e", e=CRI)
                    gp = ps2.tile([128, 2 * G], F32, tag="psb")
                    gr = gp[:, 0:G]
                    gi = gp[:, G:2 * G]
                    for yt in range(2):
                        av = artT[yt][:, vt * 128:(vt + 1) * 128]
                        aiv = aitT[yt][:, vt * 128:(vt + 1) * 128]
                        ainv = aitnT[yt][:, vt * 128:(vt + 1) * 128]
                        btr = bt[(0, yt)]
                        bti = bt[(1, yt)]
                        st = yt == 0
                        sp = yt == 1
                        nc.tensor.matmul(gr, av, btr[:], start=st, stop=False)
                        nc.tensor.matmul(gi, aiv, btr[:], start=st, stop=False)
                        nc.tensor.matmul(gr, ainv, bti[:], start=False, stop=sp)
                        nc.tensor.matmul(gi, av, bti[:], start=False, stop=sp)
                    # fused drain: [u, (re, im)] pairs, contiguous 2-element
                    # writes into the 16-stride cri interleave
                    _drain(
                        stg3[:, :, 2 * c:2 * c + 2],
                        gp[:].rearrange("p (ri u) -> p u ri", ri=2),
                    )

            # =============== phase B: shift + interleave + store ===============
            t_stores = []
            all_vt_stores = []
            for vt in range(4):
                halo = workp.tile([128, 4 * CELL], BF16, tag=f"halo{vt}")
                halo4 = halo[:].rearrange("p (c s e) -> p c s e", s=TAPS, e=CRI)
                for h in range(4):
                    rc = rowcp.tile([128, 128 * CELL], BF16, tag="rc")
                    rc3 = rc[:].rearrange("p (q e) -> p q e", e=CELL)
                    for s in range(TAPS):
                        d = s - 2
                        for sub in range(4):
                            u0 = h * 128 + sub * 32
                            src = stgs[vt][:, u0 * CRI:(u0 + 32) * CRI]
                            if d == 0:
                                sview = src.rearrange("p (q e) -> p q e", e=CRI)
                                _drain(
                                    rc3[:, sub * 32:(sub + 1) * 32,
                                        s * CRI:(s + 1) * CRI],
                                    sview,
                                )
                                if h == 3 and sub == 3:
                                    _drain(halo4[:, 0:2, s, :], sview[:, 30:32, :])
                                if h == 0 and sub == 0:
                                    _drain(halo4[:, 2:4, s, :], sview[:, 0:2, :])
                                continue
                            ps = ps1.tile([128, 512], F32, tag="psa")
                            nc.tensor.matmul(ps[:], shm[d][:], src,
                                             start=True, stop=True)
                            pview = ps[:].rearrange("p (q e) -> p q e", e=CRI)
                            _drain(
                                rc3[:, sub * 32:(sub + 1) * 32,
                                    s * CRI:(s + 1) * CRI],
                                pview,
                            )
                            if h == 3 and sub == 3:
                                _drain(halo4[:, 0:2, s, :], pview[:, 30:32, :])
                            if h == 0 and sub == 0:
                                _drain(halo4[:, 2:4, s, :], pview[:, 0:2, :])
                    t_stores.append(nc.sync.dma_start(
                        out=T2[vt * 128:(vt + 1) * 128,
                               (h * 128 + 2) * CELL:(h * 128 + 130) * CELL],
                        in_=rc[:],
                    ))
                # halo cells: q 0,1 <- u 510,511 ; q 514,515 <- u 0,1
                vt_stores = t_stores[-4:]
                vt_stores.append(nc.sync.dma_start(
                    out=T2[vt * 128:(vt + 1) * 128, 0:2 * CELL],
                    in_=halo[:, 0:2 * CELL],
                ))
                vt_stores.append(nc.sync.dma_start(
                    out=T2[vt * 128:(vt + 1) * 128, 514 * CELL:516 * CELL],
                    in_=halo[:, 2 * CELL:4 * CELL],
                ))
                t_stores += vt_stores[-2:]
                all_vt_stores.append(vt_stores)

            # edge-row fixups: the main shift matmul leaves the |d| rows that
            # wrap into the neighbouring v-tile zero; overwrite them with small
            # strided writes straight from stg.  Issued from the (idle) Pool
            # engine queue so they don't serialize the sync-queue stores.
            for vt in range(4):
                for s in (0, 1, 3, 4):
                    d = s - 2
                    if d > 0:
                        r0, n = 128 - d, d
                        nb_t = stgs[(vt + 1) % 4]
                        v0 = 0
                    else:
                        r0, n = 0, -d
                        nb_t = stgs[(vt - 1) % 4]
                        v0 = 128 + d
                    rg = vt * 128 + r0
                    src = nb_t[v0:v0 + n, :].rearrange("p (u e) -> p u e", e=CRI)
                    o3 = T2[rg:rg + n, :].rearrange("p (q e) -> p q e", e=CELL)
                    fx = [
                        nc.gpsimd.dma_start(
                            out=o3[:, 2:514, s * CRI:(s + 1) * CRI], in_=src),
                        nc.gpsimd.dma_start(
                            out=o3[:, 0:2, s * CRI:(s + 1) * CRI],
                            in_=src[:, 510:512, :]),
                        nc.gpsimd.dma_start(
                            out=o3[:, 514:516, s * CRI:(s + 1) * CRI],
                            in_=src[:, 0:2, :]),
                    ]
                    for f_ in fx:
                        for st_ in all_vt_stores[vt]:
                            tile.add_dep_helper(f_.ins, st_.ins, reason="fix WAW")
                    t_stores += fx

            # =============== phase C: gather + combine ===============
            tab_flat = T2[:].rearrange("r (q e) -> (r q) e", e=CELL)
            all_gathers = []
            for t in range(NTILE):
                w25b = w25all[:, t * 200:(t + 1) * 200]
                patch = patchp.tile([128, GRP * TAPS * CELL], BF16, tag="patch")
                for g in range(GRP):
                    col = t * GRP + g
                    gi_ = nc.gpsimd.indirect_dma_start(
                        out=patch[:, g * TAPS * CELL:(g + 1) * TAPS * CELL],
                        out_offset=None,
                        in_=tab_flat,
                        in_offset=bass.IndirectOffsetOnAxis(
                            ap=idx32[:, col:col + 1], axis=0
                        ),
                    )
                    all_gathers.append(gi_)
                # WP[p, (g, cr, ab)] = patch[p, (g, a, b, cr)] * W25
                wp = wpp.tile([128, GRP * TAPS * CELL], BF16, tag="wpt")
                pv = bass.AP(
                    patch[:].tensor, patch[:].offset,
                    [patch[:].ap[0],
                     [TAPS * CELL, GRP], [1, CRI], [CRI, TAPS * TAPS]],
                )
                wv = bass.AP(
                    w25b.tensor, w25b.offset,
                    [w25b.ap[0],
                     [TAPS * TAPS, GRP], [0, CRI], [1, TAPS * TAPS]],
                )
                ov = bass.AP(
                    wp[:].tensor, wp[:].offset,
                    [wp[:].ap[0],
                     [TAPS * CELL, GRP], [TAPS * TAPS, CRI], [1, TAPS * TAPS]],
                )
                nc.vector.tensor_tensor(out=ov, in0=pv, in1=wv, op=OP.mult)
                # reduce innermost (a,b)=25 -> res[:, t*128 + g*16 + cr]
                rv = bass.AP(
                    res[:].tensor, res[:].offset + t * 128,
                    [res[:].ap[0], [CRI, GRP], [1, CRI]],
                )
                wp3 = wp[:].rearrange("p (g cr ab) -> p g cr ab",
                                      cr=CRI, ab=TAPS * TAPS)
                with nc.allow_low_precision(
                        reason="25-tap bf16 sum; table is bf16 already"):
                    nc.vector.tensor_reduce(out=rv, in_=wp3, axis=AX.X,
                                            op=OP.add)

            # explicit RAW edges: gathers after table stores
            for gi_ in all_gathers:
                for si in t_stores:
                    tile.add_dep_helper(gi_.ins, si.ins, reason="T2 RAW")

            # ======== sqrt(w) scale + store (in-place into wsq) ========
            nc.vector.tensor_mul(wsq[:], res[:], wsq[:])
            nc.sync.dma_start(out=y_out[:], in_=wsq[:])

            if debug:
                dbg_outs = {
                    "kgo": kg, "acco": acc, "idxo": idx32, "flo": flm, "ffo": ff,
                }
                for nm, t_ in dbg_outs.items():
                    o = nc.dram_tensor(nm, list(t_[:].shape), t_[:].dtype,
                                       kind="ExternalOutput")
                    nc.sync.dma_start(out=o[:], in_=t_[:])
                o = nc.dram_tensor("t2o", [ROWS, QC * CELL], BF16,
                                   kind="ExternalOutput")
                di = nc.sync.dma_start(out=o[:], in_=T2[:])
                for si in t_stores:
                    tile.add_dep_helper(di.ins, si.ins, reason="T2 dump RAW")

    nc.compile()
    return nc


_NC_CACHE = None


def _get_nc():
    global _NC_CACHE
    if _NC_CACHE is None:
        _NC_CACHE = build_bass()
    return _NC_CACHE


# ---------------------------------------------------------------- host glue
def _shuffle_w(w_t):
    # w[c, ri, K] -> [p, (t, g, c, ri)] with K = t*1024 + g*128 + p
    v = w_t.reshape(NC, 2, NTILE, GRP, 128)
    return np.ascontiguousarray(v.transpose(4, 2, 3, 0, 1).reshape(128, NTILE * 128))


def _unshuffle_y(yr):
    # [p, (t, g, c, ri)] -> y[c, ri, K]
    v = yr.reshape(128, NTILE, GRP, NC, 2)
    return np.ascontiguousarray(v.transpose(3, 4, 1, 2, 0).reshape(NC, 2, K))


def make_in_maps(x, k, coil_sensitivities, w):
    in_maps = []
    coil0 = np.ascontiguousarray(coil_sensitivities[0], dtype=np.float32)
    for t in range(NT):
        in_maps.append({
            "x": np.ascontiguousarray(x[t], dtype=np.float32),
            "kk": np.ascontiguousarray(k[t], dtype=np.float32),
            "coil": coil0,
            "wr": _shuffle_w(np.asarray(w[t], dtype=np.float32)),
            "art": _ART, "ait": _AIT, "aitn": _AITN, "shifts": _SHIFTS,
        })
    return in_maps


def run(x, k, coil_sensitivities, w, trace=False, **spmd_kwargs):
    nc = _get_nc()
    in_maps = make_in_maps(x, k, coil_sensitivities, w)
    r = run_bass_kernel_spmd(nc, in_maps, list(range(NT)), trace=trace, **spmd_kwargs)
    y = np.stack([_unshuffle_y(r.results[t]["yr"]) for t in range(NT)], axis=0)
    return y.astype(np.float32), r


def kernel(x, k, coil_sensitivities, w):
    y, _ = run(x, k, coil_sensitivities, w, trace=False)
    return y
